# Initial kernel scaffold
#
"""TRN2 Bass kernel for nn_MultiHeadAttention (B=4, S=2048, D=1024, H=16).

Sharding: 8 cores = (batch b, query-half jq). Each core computes the full
attention for its 1024-query slice of batch b: QKV projections, 16-head
softmax attention over all 2048 keys, output projection. Outputs are
disjoint slices of the final tensor -> no cross-core reduction.

Per-core dataflow (all matmuls in float32r: fp32 bits, PE fast mode):
  A2: V = value @ Wv.T          -> V_aug [Sk, 16*(64+1)] spilled to DRAM
      (per-head 64 cols + a ones column; key_padding_mask folded in as a
       0/1 per-key row scale, which also masks the softmax denominator)
  A1: K^T = (key @ Wk.T).T      -> [D, Sk] spilled to DRAM (per dout tile)
  B(pair p of 2 heads): Q^T pair projected on the fly; S^T = K^T.T-slices
      against Q^T (row-tiled 2x: head0 on PE rows 0-63, head1 on 64-127);
      E^T = exp(S^T/8) on ScalarE straight out of PSUM;
      O^T_unnorm/sums = [V_h|1].T @ E^T accumulated over Sk (M=65);
      normalize with DVE reciprocal + GpSimd partition broadcast.
  C:  out = O^T.T @ Wo.T + bo
"""

import numpy as np

import concourse.bass as bass
import concourse.mybir as mybir
import concourse.tile as tile
from concourse import bacc
from concourse.bass_utils import run_bass_kernel_spmd

F32 = mybir.dt.float32
F32R = mybir.dt.float32r
F16 = mybir.dt.float16
EXP = mybir.ActivationFunctionType.Exp
ADD = mybir.AluOpType.add
DIV = mybir.AluOpType.divide

# Problem dims (hardcoded per harness contract)
B, S, D = 4, 2048, 1024
H, DK = 16, 64
SQ = 1024  # queries per core
SK = 2048
P = 128
CH = D // P  # 8 contraction chunks
NP_ = H // 2  # 8 head pairs
SCALE = 1.0 / np.sqrt(DK)

# Tuning knobs
QT = 512  # query tile in phase B
NQ = SQ // QT
AG = 2  # sk tiles per QK psum group (exp batch)
NKT = SK // P  # 16 sk tiles
PAIR_QK = False

ds = bass.ds


def build_nc():
    nc = bacc.Bacc("TRN2", target_bir_lowering=False, debug=False)

    qT_d = nc.dram_tensor("qT", [D, SQ], F16, kind="ExternalInput").ap()
    kT_d = nc.dram_tensor("kT", [D, SK], F16, kind="ExternalInput").ap()
    vT_d = nc.dram_tensor("vT", [D, SK], F16, kind="ExternalInput").ap()
    wq_d = nc.dram_tensor("wq", [D, D], F16, kind="ExternalInput").ap()
    wk_d = nc.dram_tensor("wk", [D, D], F16, kind="ExternalInput").ap()
    wv_d = nc.dram_tensor("wv", [D, D], F16, kind="ExternalInput").ap()
    wo_d = nc.dram_tensor("wo", [D, D], F16, kind="ExternalInput").ap()
    bo_d = nc.dram_tensor("bo", [P, D], F32, kind="ExternalInput").ap()
    mask_d = nc.dram_tensor("mask", [P, NKT], F32, kind="ExternalInput").ap()
    out_d = nc.dram_tensor("out", [SQ, D], F32, kind="ExternalOutput").ap()

    with tile.TileContext(nc) as tc:
        with (
            tc.tile_pool(name="gpool", bufs=1) as gpool,
            tc.tile_pool(name="pspool", bufs=2, space="PSUM") as pspool,
            tc.tile_pool(name="pso_pool", bufs=3, space="PSUM") as pso_pool,
            tc.tile_pool(name="dpool", bufs=1, space="DRAM") as dpool,
        ):
            mask_t = gpool.tile([P, NKT], F32, tag="mask")
            nc.sync.dma_start(mask_t[:], mask_d[:])
            oT = gpool.tile([P, CH, SQ], F16, tag="oT")

            va_sp = dpool.tile([SK, H * 65], F16, tag="va_sp")
            kt_sp = dpool.tile([D, SK], F16, tag="kt_sp")

            # ---- Phase A2: V_aug = [value @ Wv.T | ones], masked ----
            with (
                tc.tile_pool(name="pa2", bufs=1) as pa2,
                tc.tile_pool(name="stg2", bufs=3) as stg2,
            ):
                vT_t = pa2.tile([P, CH, SK], F16, tag="vT")
                nc.sync.dma_start(vT_t[:], vT_d.rearrange("(c p) s -> p c s", p=P))
                wv_t = pa2.tile([P, CH, D], F16, tag="wv")
                nc.sync.dma_start(wv_t[:], wv_d.rearrange("(c p) n -> p c n", p=P))
                for nh in range(2):  # dout halves = heads 8*nh .. 8*nh+7
                    for m in range(NKT):  # sk tiles
                        ps = pspool.tile([P, 512], F32, tag="ps_s")
                        for c in range(CH):
                            nc.tensor.matmul(
                                ps[:],
                                vT_t[:, c, ds(m * P, P)],
                                wv_t[:, c, ds(nh * 512, 512)],
                                start=(c == 0),
                                stop=(c == CH - 1),
                            )
                        st = stg2.tile([P, 8, 65], F16, tag="va")
                        nc.vector.tensor_scalar_mul(
                            st[:, :, 0:64],
                            ps[:].rearrange("p (a b) -> p a b", a=8),
                            mask_t[:, ds(m, 1)],
                        )
                        nc.vector.tensor_copy(
                            st[:, :, 64], mask_t[:, ds(m, 1)].to_broadcast([P, 8])
                        )
                        nc.sync.dma_start(
                            va_sp[ds(m * P, P), ds(nh * 520, 520)],
                            st[:].rearrange("p a b -> p (a b)"),
                        )

            # ---- Phase A1: K^T spilled per dout tile ----
            with (
                tc.tile_pool(name="pa1", bufs=1) as pa1,
                tc.tile_pool(name="stg1", bufs=3) as stg1,
            ):
                kT_t = pa1.tile([P, CH, SK], F16, tag="kT")
                nc.sync.dma_start(kT_t[:], kT_d.rearrange("(c p) s -> p c s", p=P))
                wk_t = pa1.tile([P, CH, D], F16, tag="wk")
                nc.sync.dma_start(wk_t[:], wk_d.rearrange("(c p) n -> p c n", p=P))
                for p_ in range(NP_):
                    for ns in range(SK // 512):
                        ps = pspool.tile([P, 512], F32, tag="ps_s")
                        for c in range(CH):
                            nc.tensor.matmul(
                                ps[:],
                                wk_t[:, c, ds(p_ * P, P)],
                                kT_t[:, c, ds(ns * 512, 512)],
                                start=(c == 0),
                                stop=(c == CH - 1),
                            )
                        st = stg1.tile([P, 512], F16, tag="kt")
                        nc.vector.tensor_copy(st[:], ps[:])
                        nc.sync.dma_start(
                            kt_sp[ds(p_ * P, P), ds(ns * 512, 512)], st[:]
                        )

            # ---- Phase B: per head pair ----
            with (
                tc.tile_pool(name="pb", bufs=1) as pb,
                tc.tile_pool(name="bpool", bufs=2) as bpool,
                tc.tile_pool(name="epool", bufs=2) as epool,
                tc.tile_pool(name="npool", bufs=2) as npool,
            ):
                qT_t = pb.tile([P, CH, SQ], F16, tag="qT")
                nc.sync.dma_start(qT_t[:], qT_d.rearrange("(c p) s -> p c s", p=P))
                wq_t = pb.tile([P, CH, D], F16, tag="wq")
                nc.sync.dma_start(wq_t[:], wq_d.rearrange("(c p) n -> p c n", p=P))

                for p_ in range(NP_):
                    ktp = bpool.tile([P, SK], F16, tag="ktp")
                    nc.sync.dma_start(ktp[:], kt_sp[ds(p_ * P, P), :])
                    vap = bpool.tile([P, NKT, 130], F16, tag="vap")
                    nc.sync.dma_start(
                        vap[:],
                        va_sp.rearrange("(t p) n -> p t n", p=P)[
                            :, :, ds(p_ * 130, 130)
                        ],
                    )
                    # A3: project Q^T pair slice
                    qtp = bpool.tile([P, SQ], F16, tag="qtp")
                    for ns in range(SQ // 512):
                        ps = pspool.tile([P, 512], F32, tag="ps_s")
                        for c in range(CH):
                            nc.tensor.matmul(
                                ps[:],
                                wq_t[:, c, ds(p_ * P, P)],
                                qT_t[:, c, ds(ns * 512, 512)],
                                start=(c == 0),
                                stop=(c == CH - 1),
                            )
                        nc.vector.tensor_copy(qtp[:, ds(ns * 512, 512)], ps[:])

                    def do_pv(e0, e1, qt, p_=p_, vap=vap):
                        for h, (e, r0) in enumerate(((e0, 0), (e1, 64))):
                            pso = pso_pool.tile([P, QT], F32, tag="pso")
                            for sk in range(NKT):
                                nc.tensor.matmul(
                                    pso[0:65, :],
                                    vap[:, sk, ds(h * 65, 65)],
                                    e[:, sk, :],
                                    start=(sk == 0),
                                    stop=(sk == NKT - 1),
                                )
                            rec = npool.tile([P, QT], F32, tag="rec")
                            rb = npool.tile([P, QT], F32, tag="rb")
                            nc.vector.reciprocal(rec[0:1, :], pso[64:65, :])
                            nc.gpsimd.partition_broadcast(rb[0:64, :], rec[0:1, :])
                            nc.vector.tensor_mul(
                                out=oT[ds(r0, 64), p_, ds(qt * QT, QT)],
                                in0=pso[0:64, :],
                                in1=rb[0:64, :],
                            )

                    prev = None
                    for qt in range(NQ):
                        e0 = epool.tile([P, NKT, QT], F16, tag="e0")
                        e1 = epool.tile([P, NKT, QT], F16, tag="e1")
                        qsl = ds(qt * QT, QT)
                        for g in range(NKT // AG):
                            ps0 = pspool.tile([P, AG, QT], F32, tag="ps_s")
                            ps1 = pspool.tile([P, AG, QT], F32, tag="ps_s")
                            for j in range(AG):
                                sk = g * AG + j
                                ksl = ds(sk * P, P)
                                nc.tensor.matmul(
                                    ps0[:, j, :],
                                    ktp[0:64, ksl],
                                    qtp[0:64, qsl],
                                    start=True,
                                    stop=True,
                                    tile_position=(0, 0) if PAIR_QK else None,
                                )
                                nc.tensor.matmul(
                                    ps1[:, j, :],
                                    ktp[64:128, ksl],
                                    qtp[64:128, qsl],
                                    start=True,
                                    stop=True,
                                    tile_position=(64, 0) if PAIR_QK else None,
                                )
                            gsl = ds(g * AG, AG)
                            nc.scalar.activation(
                                e0[:, gsl, :], ps0[:], EXP, scale=SCALE
                            )
                            nc.scalar.activation(
                                e1[:, gsl, :], ps1[:], EXP, scale=SCALE
                            )
                        if prev is not None:
                            do_pv(*prev)
                        prev = (e0, e1, qt)
                    do_pv(*prev)

            # ---- Phase C: out = O^T.T @ Wo.T + bo ----
            with (
                tc.tile_pool(name="pc", bufs=1) as pc,
                tc.tile_pool(name="stgc", bufs=3) as stgc,
            ):
                wo_t = pc.tile([P, CH, D], F16, tag="wo")
                nc.sync.dma_start(wo_t[:], wo_d.rearrange("(c p) n -> p c n", p=P))
                bo_t = pc.tile([P, D], F32, tag="bo")
                nc.sync.dma_start(bo_t[:], bo_d[:])
                for m in range(SQ // P):
                    for nh in range(2):
                        ps = pspool.tile([P, 512], F32, tag="ps_s")
                        for c in range(CH):
                            nc.tensor.matmul(
                                ps[:],
                                oT[:, c, ds(m * P, P)],
                                wo_t[:, c, ds(nh * 512, 512)],
                                start=(c == 0),
                                stop=(c == CH - 1),
                            )
                        st = stgc.tile([P, 512], F32, tag="co")
                        nc.vector.tensor_tensor(
                            st[:], ps[:], bo_t[:, ds(nh * 512, 512)], ADD
                        )
                        nc.sync.dma_start(
                            out_d[ds(m * P, P), ds(nh * 512, 512)], st[:]
                        )

    nc.compile()
    return nc


_NC = None


def _get_nc():
    global _NC
    if _NC is None:
        _NC = build_nc()
    return _NC


def make_in_maps(query, key, value, key_padding_mask, Wq, Wk, Wv, Wo, bo):
    query = np.asarray(query, dtype=np.float16)
    key = np.asarray(key, dtype=np.float16)
    value = np.asarray(value, dtype=np.float16)
    mask = np.asarray(key_padding_mask)
    wq_t = np.ascontiguousarray(np.asarray(Wq, dtype=np.float16).T)
    wk_t = np.ascontiguousarray(np.asarray(Wk, dtype=np.float16).T)
    wv_t = np.ascontiguousarray(np.asarray(Wv, dtype=np.float16).T)
    wo_t = np.ascontiguousarray(np.asarray(Wo, dtype=np.float16).T)
    bo_rep = np.ascontiguousarray(
        np.broadcast_to(np.asarray(bo, dtype=np.float32), (P, D))
    )
    in_maps = []
    for core in range(8):
        b, jq = core // 2, core % 2
        in_maps.append(
            {
                "qT": np.ascontiguousarray(query[b, jq * SQ : (jq + 1) * SQ, :].T),
                "kT": np.ascontiguousarray(key[b].T),
                "vT": np.ascontiguousarray(value[b].T),
                "wq": wq_t,
                "wk": wk_t,
                "wv": wv_t,
                "wo": wo_t,
                "bo": bo_rep,
                "mask": np.ascontiguousarray(
                    mask[b].astype(np.float32).reshape(NKT, P).T
                ),
            }
        )
    return in_maps


def run_sharded(inputs, trace=False, trace_cores=None):
    nc = _get_nc()
    in_maps = make_in_maps(**inputs)
    res = run_bass_kernel_spmd(
        nc,
        in_maps,
        list(range(8)),
        trace=trace,
        trace_cores=trace_cores,
    )
    full = np.empty((B, S, D), dtype=np.float32)
    for core in range(8):
        b, jq = core // 2, core % 2
        full[b, jq * SQ : (jq + 1) * SQ, :] = res.results[core]["out"]
    return full, res


def kernel(**inputs):
    full, _ = run_sharded(inputs)
    return full



# revision 4
# speedup vs baseline: 1.6929x; 1.6929x over previous
"""TRN2 Bass kernel for nn_MultiHeadAttention (B=4, S=2048, D=1024, H=16).

Sharding: 8 cores = (batch b, query-half jq). Each core computes the full
attention for its 1024-query slice of batch b: QKV projections, 16-head
softmax attention over all 2048 keys, output projection. Outputs are
disjoint slices of the final tensor -> no cross-core reduction.

v3 design (HAM-warm, SBUF-resident, normalize off the PSUM critical path):
  A2: V_aug = [value @ Wv.T | ones]*mask -> va_f SBUF [128, 16, 16*65]
  A1: K^T = (key @ Wk.T).T             -> kt_f SBUF [128, 8, 2048]
  B:  16 units = (pair p, query-tile qt of 512). Per unit, one dense PE
      stream: paired QK via tile_position row-tiling (head0 rows 0-63,
      head1 rows 64-127, concurrent), ScalarE exp straight out of PSUM
      (both heads, one inst per sk tile), PV accumulation for unit i-1
      and Q-projection for unit i+1 interleaved between QK groups. PV
      PSUM is immediately copied (unnormalized, with the denominator
      row) to SBUF so the PSUM bank frees in ~0.3us; the reciprocal/
      broadcast/multiply normalization runs from SBUF off-path.
  C:  out = O^T.T @ Wo.T + bo
Input DMAs are chunked and issue-ordered to match consumption.
"""

import numpy as np

import concourse.bass as bass
import concourse.mybir as mybir
import concourse.tile as tile
from concourse import bacc
from concourse.bass_utils import run_bass_kernel_spmd

F32 = mybir.dt.float32
F16 = mybir.dt.float16
EXP = mybir.ActivationFunctionType.Exp
ADD = mybir.AluOpType.add

# Problem dims (hardcoded per harness contract)
B, S, D = 4, 2048, 1024
H, DK = 16, 64
SQ = 1024  # queries per core
SK = 2048
P = 128
CH = D // P  # 8 contraction chunks
NP_ = H // 2  # 8 head pairs
SCALE = 1.0 / np.sqrt(DK)

QT = 512  # query tile in phase B
NQ = SQ // QT
NKT = SK // P  # 16 sk tiles
PAIR_QK = True
EBUFS = 20

ds = bass.ds


def build_nc():
    nc = bacc.Bacc("TRN2", target_bir_lowering=False, debug=False)

    qT_d = nc.dram_tensor("qT", [D, SQ], F16, kind="ExternalInput").ap()
    kT_d = nc.dram_tensor("kT", [D, SK], F16, kind="ExternalInput").ap()
    vT_d = nc.dram_tensor("vT", [D, SK], F16, kind="ExternalInput").ap()
    wq_d = nc.dram_tensor("wq", [D, D], F16, kind="ExternalInput").ap()
    wk_d = nc.dram_tensor("wk", [D, D], F16, kind="ExternalInput").ap()
    wv_d = nc.dram_tensor("wv", [D, D], F16, kind="ExternalInput").ap()
    wo_d = nc.dram_tensor("wo", [D, D], F16, kind="ExternalInput").ap()
    bo_d = nc.dram_tensor("bo", [P, D], F32, kind="ExternalInput").ap()
    mask_d = nc.dram_tensor("mask", [P, NKT], F32, kind="ExternalInput").ap()
    out_d = nc.dram_tensor("out", [SQ, D], F32, kind="ExternalOutput").ap()

    with tile.TileContext(nc) as tc:
        with (
            tc.tile_pool(name="gpool", bufs=1) as gpool,
        ):
            mask_t = gpool.tile([P, NKT], F32, tag="mask")
            nc.sync.dma_start(mask_t[:], mask_d[:])
            oT = gpool.tile([P, CH, SQ], F16, tag="oT")
            # SBUF-resident K^T and V_aug (no DRAM spill)
            kt_f = gpool.tile([P, NP_, SK], F16, tag="kt_f")
            va_f = gpool.tile([P, NKT, H * 65], F16, tag="va_f")
            # Query input + Wq live in gpool; DMAs issued during A1.
            qT_t = gpool.tile([P, CH, SQ], F16, tag="qT")
            wq_t = gpool.tile([P, CH, D], F16, tag="wq")

            # ---- Phase A2: V_aug = [value @ Wv.T | ones], masked ----
            with (
                tc.tile_pool(name="pa2", bufs=1) as pa2,
                tc.tile_pool(name="psa2", bufs=3, space="PSUM") as psa2,
            ):
                wv_t = pa2.tile([P, CH, D], F16, tag="wv")
                wv_r = wv_d.rearrange("(c p) n -> p c n", p=P)
                for nh in range(2):
                    for c in range(CH):
                        nc.sync.dma_start(
                            wv_t[:, c, ds(nh * 512, 512)],
                            wv_r[:, c, ds(nh * 512, 512)],
                        )
                vT_t = pa2.tile([P, CH, SK], F16, tag="vT")
                vT_r = vT_d.rearrange("(c p) s -> p c s", p=P)
                for m in range(NKT):
                    nc.sync.dma_start(
                        vT_t[:, :, ds(m * P, P)], vT_r[:, :, ds(m * P, P)]
                    )
                for nh in range(2):  # dout halves = heads 8*nh .. 8*nh+7
                    for m in range(NKT):  # sk tiles
                        ps = psa2.tile([P, 512], F32, tag="psA")
                        for c in range(CH):
                            nc.tensor.matmul(
                                ps[:],
                                vT_t[:, c, ds(m * P, P)],
                                wv_t[:, c, ds(nh * 512, 512)],
                                start=(c == 0),
                                stop=(c == CH - 1),
                            )
                        dst = va_f[:, m, ds(nh * 520, 520)].rearrange(
                            "p (a b) -> p a b", a=8
                        )
                        nc.vector.tensor_scalar_mul(
                            dst[:, :, 0:64],
                            ps[:].rearrange("p (a b) -> p a b", a=8),
                            mask_t[:, ds(m, 1)],
                        )
                        nc.vector.tensor_copy(
                            dst[:, :, 64], mask_t[:, ds(m, 1)].to_broadcast([P, 8])
                        )

            # ---- Phase A1: K^T -> kt_f ----
            with (
                tc.tile_pool(name="pa1", bufs=1) as pa1,
                tc.tile_pool(name="psa1", bufs=3, space="PSUM") as psa1,
            ):
                wk_t = pa1.tile([P, CH, D], F16, tag="wk")
                wk_r = wk_d.rearrange("(c p) n -> p c n", p=P)
                for p_ in range(NP_):
                    nc.sync.dma_start(
                        wk_t[:, :, ds(p_ * P, P)], wk_r[:, :, ds(p_ * P, P)]
                    )
                kT_t = pa1.tile([P, CH, SK], F16, tag="kT")
                kT_r = kT_d.rearrange("(c p) s -> p c s", p=P)
                for ns in range(SK // 512):
                    nc.sync.dma_start(
                        kT_t[:, :, ds(ns * 512, 512)], kT_r[:, :, ds(ns * 512, 512)]
                    )
                # Queue Q/Wq input loads behind A1's own inputs; they land
                # well before phase B's Q projection needs them.
                qT_r = qT_d.rearrange("(c p) s -> p c s", p=P)
                for jq in range(NQ):
                    nc.sync.dma_start(
                        qT_t[:, :, ds(jq * QT, QT)], qT_r[:, :, ds(jq * QT, QT)]
                    )
                wq_r = wq_d.rearrange("(c p) n -> p c n", p=P)
                for p_ in range(NP_):
                    nc.sync.dma_start(
                        wq_t[:, :, ds(p_ * P, P)], wq_r[:, :, ds(p_ * P, P)]
                    )
                for ns in range(SK // 512):
                    for p_ in range(NP_):
                        ps = psa1.tile([P, 512], F32, tag="psA")
                        for c in range(CH):
                            nc.tensor.matmul(
                                ps[:],
                                wk_t[:, c, ds(p_ * P, P)],
                                kT_t[:, c, ds(ns * 512, 512)],
                                start=(c == 0),
                                stop=(c == CH - 1),
                            )
                        nc.vector.tensor_copy(
                            kt_f[:, p_, ds(ns * 512, 512)], ps[:]
                        )

            # ---- Phase B: 16 pipelined units of (pair, query-tile) ----
            with (
                tc.tile_pool(name="qtpool", bufs=2) as qtpool,
                tc.tile_pool(name="epool", bufs=EBUFS) as epool,
                tc.tile_pool(name="npool", bufs=2) as npool,
                tc.tile_pool(name="qkps", bufs=2, space="PSUM") as qkps,
                tc.tile_pool(name="psop", bufs=2, space="PSUM") as psop,
                tc.tile_pool(name="qpps", bufs=2, space="PSUM") as qpps,
            ):
                units = [(p_, qt) for p_ in range(NP_) for qt in range(NQ)]
                NU = len(units)

                def qproj_mm(qp, unit, c):
                    p_, qt = unit
                    nc.tensor.matmul(
                        qp[:],
                        wq_t[:, c, ds(p_ * P, P)],
                        qT_t[:, c, ds(qt * QT, QT)],
                        start=(c == 0),
                        stop=(c == CH - 1),
                    )

                def pv_mms(pso_pair, unit, sk, e_sk):
                    p_, qt = unit
                    for hh in range(2):
                        nc.tensor.matmul(
                            pso_pair[hh][0:65, :],
                            va_f[:, sk, ds((p_ * 2 + hh) * 65, 65)],
                            e_sk[:, hh, :],
                            start=(sk == 0),
                            stop=(sk == NKT - 1),
                        )

                def spill_o(pso_pair):
                    ou = npool.tile([P, 2, QT], F32, tag="ou", name="ou")
                    for hh in range(2):
                        nc.vector.tensor_copy(
                            ou[0:65, hh, :], pso_pair[hh][0:65, :]
                        )
                    return ou

                def normalize(ou, unit):
                    p_, qt = unit
                    for hh in range(2):
                        rec = npool.tile([P, QT], F32, tag="rec", name="rec")
                        rb = npool.tile([P, QT], F32, tag="rb", name="rb")
                        nc.vector.reciprocal(rec[0:1, :], ou[64:65, hh, :])
                        nc.gpsimd.partition_broadcast(rb[0:64, :], rec[0:1, :])
                        nc.vector.tensor_mul(
                            out=oT[ds(hh * 64, 64), p_, ds(qt * QT, QT)],
                            in0=ou[0:64, hh, :],
                            in1=rb[0:64, :],
                        )

                # Prelude: project Q for unit 0
                qp = qpps.tile([P, QT], F32, tag="qp")
                for c in range(CH):
                    qproj_mm(qp, units[0], c)
                qtp_cur = qtpool.tile([P, QT], F16, tag="qtp")
                nc.vector.tensor_copy(qtp_cur[:], qp[:])

                prev_e = None
                prev_unit = None
                for i, unit in enumerate(units):
                    p_, qt = unit
                    cur_e = []
                    if i >= 1:
                        pso_pair = (
                            psop.tile([P, QT], F32, tag="pso", name="pso0"),
                            psop.tile([P, QT], F32, tag="pso", name="pso1"),
                        )
                    if i + 1 < NU:
                        qp_next = qpps.tile([P, QT], F32, tag="qp")
                        qtp_next = qtpool.tile([P, QT], F16, tag="qtp")
                    for sk in range(NKT):
                        qk = qkps.tile([P, 2, QT], F32, tag="qk")
                        ksl = ds(sk * P, P)
                        nc.tensor.matmul(
                            qk[:, 0, :],
                            kt_f[0:64, p_, ksl],
                            qtp_cur[0:64, :],
                            start=True,
                            stop=True,
                            tile_position=(0, 0) if PAIR_QK else None,
                        )
                        nc.tensor.matmul(
                            qk[:, 1, :],
                            kt_f[64:128, p_, ksl],
                            qtp_cur[64:128, :],
                            start=True,
                            stop=True,
                            tile_position=(64, 0) if PAIR_QK else None,
                        )
                        e_sk = epool.tile([P, 2, QT], F16, tag="e", name="e_sk")
                        cur_e.append(e_sk)
                        nc.scalar.activation(e_sk[:], qk[:], EXP, scale=SCALE)
                        if i >= 1:
                            pv_mms(pso_pair, prev_unit, sk, prev_e[sk])
                        if i + 1 < NU and sk % 2 == 0:
                            qproj_mm(qp_next, units[i + 1], sk // 2)
                    if i >= 1:
                        ou = spill_o(pso_pair)
                    if i + 1 < NU:
                        nc.vector.tensor_copy(qtp_next[:], qp_next[:])
                    if i >= 1:
                        normalize(ou, prev_unit)
                    prev_e = cur_e
                    prev_unit = unit
                    if i + 1 < NU:
                        qtp_cur = qtp_next
                # Epilogue: PV + normalize for the last unit
                pso_pair = (
                    psop.tile([P, QT], F32, tag="pso", name="pso0"),
                    psop.tile([P, QT], F32, tag="pso", name="pso1"),
                )
                for sk in range(NKT):
                    pv_mms(pso_pair, prev_unit, sk, prev_e[sk])
                ou = spill_o(pso_pair)
                normalize(ou, prev_unit)

            # ---- Phase C: out = O^T.T @ Wo.T + bo ----
            with (
                tc.tile_pool(name="pc", bufs=1) as pc,
                tc.tile_pool(name="stgc", bufs=3) as stgc,
                tc.tile_pool(name="psc", bufs=3, space="PSUM") as psc,
            ):
                wo_t = pc.tile([P, CH, D], F16, tag="wo")
                wo_r = wo_d.rearrange("(c p) n -> p c n", p=P)
                for nh in range(2):
                    for c in range(CH):
                        nc.sync.dma_start(
                            wo_t[:, c, ds(nh * 512, 512)],
                            wo_r[:, c, ds(nh * 512, 512)],
                        )
                bo_t = pc.tile([P, D], F32, tag="bo")
                nc.sync.dma_start(bo_t[:], bo_d[:])
                for m in range(SQ // P):
                    for nh in range(2):
                        ps = psc.tile([P, 512], F32, tag="psC")
                        for c in range(CH):
                            nc.tensor.matmul(
                                ps[:],
                                oT[:, c, ds(m * P, P)],
                                wo_t[:, c, ds(nh * 512, 512)],
                                start=(c == 0),
                                stop=(c == CH - 1),
                            )
                        st = stgc.tile([P, 512], F32, tag="co")
                        nc.vector.tensor_tensor(
                            st[:], ps[:], bo_t[:, ds(nh * 512, 512)], ADD
                        )
                        nc.sync.dma_start(
                            out_d[ds(m * P, P), ds(nh * 512, 512)], st[:]
                        )

    nc.compile()
    return nc


_NC = None


def _get_nc():
    global _NC
    if _NC is None:
        _NC = build_nc()
    return _NC


def make_in_maps(query, key, value, key_padding_mask, Wq, Wk, Wv, Wo, bo):
    query = np.asarray(query, dtype=np.float16)
    key = np.asarray(key, dtype=np.float16)
    value = np.asarray(value, dtype=np.float16)
    mask = np.asarray(key_padding_mask)
    wq_t = np.ascontiguousarray(np.asarray(Wq, dtype=np.float16).T)
    wk_t = np.ascontiguousarray(np.asarray(Wk, dtype=np.float16).T)
    wv_t = np.ascontiguousarray(np.asarray(Wv, dtype=np.float16).T)
    wo_t = np.ascontiguousarray(np.asarray(Wo, dtype=np.float16).T)
    bo_rep = np.ascontiguousarray(
        np.broadcast_to(np.asarray(bo, dtype=np.float32), (P, D))
    )
    in_maps = []
    for core in range(8):
        b, jq = core // 2, core % 2
        in_maps.append(
            {
                "qT": np.ascontiguousarray(query[b, jq * SQ : (jq + 1) * SQ, :].T),
                "kT": np.ascontiguousarray(key[b].T),
                "vT": np.ascontiguousarray(value[b].T),
                "wq": wq_t,
                "wk": wk_t,
                "wv": wv_t,
                "wo": wo_t,
                "bo": bo_rep,
                "mask": np.ascontiguousarray(
                    mask[b].astype(np.float32).reshape(NKT, P).T
                ),
            }
        )
    return in_maps


def run_sharded(inputs, trace=False, trace_cores=None):
    nc = _get_nc()
    in_maps = make_in_maps(**inputs)
    res = run_bass_kernel_spmd(
        nc,
        in_maps,
        list(range(8)),
        trace=trace,
        trace_cores=trace_cores,
    )
    full = np.empty((B, S, D), dtype=np.float32)
    for core in range(8):
        b, jq = core // 2, core % 2
        full[b, jq * SQ : (jq + 1) * SQ, :] = res.results[core]["out"]
    return full, res


def kernel(**inputs):
    full, _ = run_sharded(inputs)
    return full


# revision 14
# speedup vs baseline: 1.8154x; 1.0723x over previous
"""TRN2 Bass kernel for nn_MultiHeadAttention (B=4, S=2048, D=1024, H=16).

Sharding: 8 cores = (batch b, query-half jq). Each core computes the full
attention for its 1024-query slice of batch b: QKV projections, 16-head
softmax attention over all 2048 keys, output projection. Outputs are
disjoint slices of the final tensor -> no cross-core reduction.

v4 design (phase B is ScalarE-exp-bound; everything else hides under it):
  A2 (serial): V_aug = [value @ Wv.T | ones]*mask -> va_f SBUF.
  B: 16 units = (pair p, query-tile qt of 512). Per unit, one dense PE
     stream: paired QK via tile_position row-tiling, ScalarE exp straight
     out of PSUM (both heads, one inst per sk tile), PV accumulation for
     unit i-1, plus a FIFO of filler matmul groups -- the K projection
     (kT streamed from DRAM in 1MB chunks, re-read per pair) and the Q
     projection (into SBUF-resident qtp_f) -- drained at a fixed rate so
     the PE keeps pace with ScalarE and the HAM clock gate stays warm.
     PV PSUM is immediately copied (unnormalized, with denominator row)
     to SBUF; reciprocal/broadcast/multiply normalization runs off-path.
  C: out = O^T.T @ Wo.T + bo.
"""

import contextlib

import numpy as np

import concourse.bass as bass
import concourse.mybir as mybir
import concourse.tile as tile
from concourse import bacc
from concourse.bass_utils import run_bass_kernel_spmd

F32 = mybir.dt.float32
F16 = mybir.dt.float16
EXP = mybir.ActivationFunctionType.Exp
ADD = mybir.AluOpType.add

# Problem dims (hardcoded per harness contract)
B, S, D = 4, 2048, 1024
H, DK = 16, 64
SQ = 1024  # queries per core
SK = 2048
P = 128
CH = D // P  # 8 contraction chunks
NP_ = H // 2  # 8 head pairs
SCALE = 1.0 / np.sqrt(DK)

QT = 512  # query tile in phase B
NQ = SQ // QT
NKT = SK // P  # 16 sk tiles
PAIR_QK = True
EBUFS = 18
FILL_SKS = (1, 5, 9, 13)  # sk steps that drain one filler group

ds = bass.ds


def build_nc():
    nc = bacc.Bacc("TRN2", target_bir_lowering=False, debug=False)

    qT_d = nc.dram_tensor("qT", [D, SQ], F16, kind="ExternalInput").ap()
    kT_d = nc.dram_tensor("kT", [D, SK], F16, kind="ExternalInput").ap()
    vT_d = nc.dram_tensor("vT", [D, SK], F16, kind="ExternalInput").ap()
    wq_d = nc.dram_tensor("wq", [D, D], F16, kind="ExternalInput").ap()
    wk_d = nc.dram_tensor("wk", [D, D], F16, kind="ExternalInput").ap()
    wv_d = nc.dram_tensor("wv", [D, D], F16, kind="ExternalInput").ap()
    wo_d = nc.dram_tensor("wo", [D, D], F16, kind="ExternalInput").ap()
    bo_d = nc.dram_tensor("bo", [P, D], F32, kind="ExternalInput").ap()
    mask_d = nc.dram_tensor("mask", [P, NKT], F32, kind="ExternalInput").ap()
    out_d = nc.dram_tensor("out", [SQ, D], F32, kind="ExternalOutput").ap()

    kT_r = kT_d.rearrange("(c p) s -> p c s", p=P)
    wk_r = wk_d.rearrange("(c p) n -> p c n", p=P)

    with tile.TileContext(nc) as tc:
        with (
            tc.tile_pool(name="gpool", bufs=1) as gpool,
            tc.tile_pool(name="ktcpool", bufs=2) as ktcpool,
            tc.tile_pool(name="wkcpool", bufs=2) as wkcpool,
        ):
            mask_t = gpool.tile([P, NKT], F32, tag="mask")
            nc.sync.dma_start(mask_t[:], mask_d[:])
            kt_f = gpool.tile([P, NP_, SK], F16, tag="kt_f")
            va_f = gpool.tile([P, NKT, H * 65], F16, tag="va_f")
            qT_t = gpool.tile([P, CH, SQ], F16, tag="qT")
            wq_t = gpool.tile([P, CH, D], F16, tag="wq")

            # ---- Phase A2 (serial): V_aug -> va_f ----
            with (
                tc.tile_pool(name="pa2", bufs=1) as pa2,
                tc.tile_pool(name="psa2", bufs=3, space="PSUM") as psa2,
            ):
                wv_t = pa2.tile([P, CH, D], F16, tag="wv")
                wv_r = wv_d.rearrange("(c p) n -> p c n", p=P)
                nc.sync.dma_start(wv_t[:, :, 0:512], wv_r[:, :, 0:512])
                vT_t = pa2.tile([P, CH, SK], F16, tag="vT")
                vT_r = vT_d.rearrange("(c p) s -> p c s", p=P)
                nc.sync.dma_start(vT_t[:, :, 0:256], vT_r[:, :, 0:256])
                nc.sync.dma_start(wv_t[:, :, 512:1024], wv_r[:, :, 512:1024])
                for mm2 in range(1, 8):
                    nc.sync.dma_start(
                        vT_t[:, :, ds(mm2 * 256, 256)],
                        vT_r[:, :, ds(mm2 * 256, 256)],
                    )
                # Pre-land pair-0 K chunks + weights + Q inputs during A2.
                kc_pre = [
                    ktcpool.tile([P, CH, 512], F16, tag="ktc", name="kc_pre")
                    for _ in range(2)
                ]
                for j, kc in enumerate(kc_pre):
                    nc.sync.dma_start(kc[:], kT_r[:, :, ds(j * 512, 512)])
                wkc_pre = wkcpool.tile([P, CH, P], F16, tag="wkc", name="wkc_pre")
                nc.sync.dma_start(wkc_pre[:], wk_r[:, :, 0:P])
                qT_r = qT_d.rearrange("(c p) s -> p c s", p=P)
                for jq in range(2):
                    nc.sync.dma_start(
                        qT_t[:, :, ds(jq * QT, QT)], qT_r[:, :, ds(jq * QT, QT)]
                    )
                wq_r = wq_d.rearrange("(c p) n -> p c n", p=P)
                for half in range(2):
                    nc.sync.dma_start(
                        wq_t[:, :, ds(half * 512, 512)],
                        wq_r[:, :, ds(half * 512, 512)],
                    )
                for nh in range(2):  # dout halves = heads 8*nh .. 8*nh+7
                    for m in range(NKT):  # sk tiles
                        ps = psa2.tile([P, 512], F32, tag="psA2")
                        for c in range(CH):
                            nc.tensor.matmul(
                                ps[:],
                                vT_t[:, c, ds(m * P, P)],
                                wv_t[:, c, ds(nh * 512, 512)],
                                start=(c == 0),
                                stop=(c == CH - 1),
                            )
                        dst = va_f[:, m, ds(nh * 520, 520)].rearrange(
                            "p (a b) -> p a b", a=8
                        )
                        nc.vector.tensor_scalar_mul(
                            dst[:, :, 0:64],
                            ps[:].rearrange("p (a b) -> p a b", a=8),
                            mask_t[:, ds(m, 1)],
                        )
                        nc.vector.tensor_copy(
                            dst[:, :, 64], mask_t[:, ds(m, 1)].to_broadcast([P, 8])
                        )

            # ---- Phase B (+ hidden K/Q projections) ----
            with (
                tc.tile_pool(name="bcpool", bufs=1) as bcpool,
                tc.tile_pool(name="psf", bufs=2, space="PSUM") as psf,
            ):
                oT = bcpool.tile([P, CH, SQ], F16, tag="oT")
                qtp_f = bcpool.tile([P, NP_, SQ], F16, tag="qtp_f")
                _bstk = contextlib.ExitStack()
                epool = _bstk.enter_context(tc.tile_pool(name="epool", bufs=EBUFS))
                npool = _bstk.enter_context(tc.tile_pool(name="npool", bufs=1))
                qkps = _bstk.enter_context(
                    tc.tile_pool(name="qkps", bufs=2, space="PSUM")
                )
                psop = _bstk.enter_context(
                    tc.tile_pool(name="psop", bufs=2, space="PSUM")
                )

                units = [(p_, qt) for p_ in range(NP_) for qt in range(NQ)]
                NU = len(units)

                wkc_cur = {0: wkc_pre}

                def a1_group(p_, ns):
                    def go():
                        if ns == 0 and p_ > 0:
                            wkc = wkcpool.tile(
                                [P, CH, P], F16, tag="wkc", name="wkc"
                            )
                            nc.sync.dma_start(wkc[:], wk_r[:, :, ds(p_ * P, P)])
                            wkc_cur[p_] = wkc
                        if p_ == 0 and ns < 2:
                            kc = kc_pre[ns]
                        else:
                            kc = ktcpool.tile(
                                [P, CH, 512], F16, tag="ktc", name="kc"
                            )
                            nc.sync.dma_start(kc[:], kT_r[:, :, ds(ns * 512, 512)])
                        wkc = wkc_cur[p_]
                        ps = psf.tile([P, 512], F32, tag="psF", name="psF")
                        for c in range(CH):
                            nc.tensor.matmul(
                                ps[:],
                                wkc[:, c, :],
                                kc[:, c, :],
                                start=(c == 0),
                                stop=(c == CH - 1),
                            )
                        nc.vector.tensor_copy(
                            kt_f[:, p_, ds(ns * 512, 512)], ps[:]
                        )

                    return go

                def qp_group(p_, qt):
                    def go():
                        ps = psf.tile([P, 512], F32, tag="psF", name="psQ")
                        for c in range(CH):
                            nc.tensor.matmul(
                                ps[:],
                                wq_t[:, c, ds(p_ * P, P)],
                                qT_t[:, c, ds(qt * QT, QT)],
                                start=(c == 0),
                                stop=(c == CH - 1),
                            )
                        nc.vector.tensor_copy(
                            qtp_f[:, p_, ds(qt * QT, QT)], ps[:]
                        )

                    return go

                fillers = []
                for p_ in range(NP_):
                    for ns in range(SK // 512):
                        fillers.append(a1_group(p_, ns))
                    for qt in range(NQ):
                        fillers.append(qp_group(p_, qt))

                # Prelude: K projection for pair 0 + Q projection (p0, qt0)
                for _ in range(5):
                    fillers.pop(0)()

                def pv_mms(pso_pair, unit, sk, e_sk):
                    p_, qt = unit
                    for hh in range(2):
                        nc.tensor.matmul(
                            pso_pair[hh][0:65, :],
                            va_f[:, sk, ds((p_ * 2 + hh) * 65, 65)],
                            e_sk[:, hh, :],
                            start=(sk == 0),
                            stop=(sk == NKT - 1),
                        )

                def spill_o(pso_pair):
                    ou = npool.tile([P, 2, QT], F32, tag="ou", name="ou")
                    for hh in range(2):
                        nc.vector.tensor_copy(
                            ou[0:65, hh, :], pso_pair[hh][0:65, :]
                        )
                    return ou

                def normalize(ou, unit):
                    p_, qt = unit
                    for hh in range(2):
                        rec = npool.tile([P, QT], F32, tag="rec", name="rec")
                        rb = npool.tile([P, QT], F32, tag="rb", name="rb")
                        nc.vector.reciprocal(rec[0:1, :], ou[64:65, hh, :])
                        nc.gpsimd.partition_broadcast(rb[0:64, :], rec[0:1, :])
                        nc.vector.tensor_mul(
                            out=oT[ds(hh * 64, 64), p_, ds(qt * QT, QT)],
                            in0=ou[0:64, hh, :],
                            in1=rb[0:64, :],
                        )

                prev_e = None
                prev_unit = None
                for i, unit in enumerate(units):
                    p_, qt = unit
                    qsl = ds(qt * QT, QT)
                    cur_e = []
                    if i >= 1:
                        pso_pair = (
                            psop.tile([P, QT], F32, tag="pso", name="pso0"),
                            psop.tile([P, QT], F32, tag="pso", name="pso1"),
                        )
                    for sk in range(NKT):
                        qk = qkps.tile([P, 2, QT], F32, tag="qk")
                        ksl = ds(sk * P, P)
                        nc.tensor.matmul(
                            qk[:, 0, :],
                            kt_f[0:64, p_, ksl],
                            qtp_f[0:64, p_, qsl],
                            start=True,
                            stop=True,
                            tile_position=(0, 0) if PAIR_QK else None,
                        )
                        nc.tensor.matmul(
                            qk[:, 1, :],
                            kt_f[64:128, p_, ksl],
                            qtp_f[64:128, p_, qsl],
                            start=True,
                            stop=True,
                            tile_position=(64, 0) if PAIR_QK else None,
                        )
                        e_sk = epool.tile([P, 2, QT], F16, tag="e", name="e_sk")
                        cur_e.append(e_sk)
                        nc.scalar.activation(e_sk[:], qk[:], EXP, scale=SCALE)
                        if i >= 1:
                            pv_mms(pso_pair, prev_unit, sk, prev_e[sk])
                        if sk in FILL_SKS and fillers:
                            fillers.pop(0)()
                    if i >= 1:
                        ou = spill_o(pso_pair)
                        normalize(ou, prev_unit)
                    prev_e = cur_e
                    prev_unit = unit
                # Epilogue: PV + normalize for the last unit
                pso_pair = (
                    psop.tile([P, QT], F32, tag="pso", name="pso0"),
                    psop.tile([P, QT], F32, tag="pso", name="pso1"),
                )
                for sk in range(NKT):
                    pv_mms(pso_pair, prev_unit, sk, prev_e[sk])
                ou = spill_o(pso_pair)
                normalize(ou, prev_unit)

                _bstk.close()  # release B-only pools before phase C
                # ---- Phase C: out = O^T.T @ Wo.T + bo ----
                with (
                    tc.tile_pool(name="pc", bufs=1) as pc,
                    tc.tile_pool(name="stgc", bufs=3) as stgc,
                ):
                    wo_t = pc.tile([P, CH, D], F16, tag="wo")
                    wo_r = wo_d.rearrange("(c p) n -> p c n", p=P)
                    for nh in range(2):
                        nc.sync.dma_start(
                            wo_t[:, :, ds(nh * 512, 512)],
                            wo_r[:, :, ds(nh * 512, 512)],
                        )
                    bo_t = pc.tile([P, D], F32, tag="bo")
                    nc.sync.dma_start(bo_t[:], bo_d[:])
                    for m in range(SQ // P):
                        for nh in range(2):
                            ps = psf.tile([P, 512], F32, tag="psF", name="psC")
                            for c in range(CH):
                                nc.tensor.matmul(
                                    ps[:],
                                    oT[:, c, ds(m * P, P)],
                                    wo_t[:, c, ds(nh * 512, 512)],
                                    start=(c == 0),
                                    stop=(c == CH - 1),
                                )
                            st = stgc.tile([P, 512], F32, tag="co")
                            nc.vector.tensor_tensor(
                                st[:], ps[:], bo_t[:, ds(nh * 512, 512)], ADD
                            )
                            nc.sync.dma_start(
                                out_d[ds(m * P, P), ds(nh * 512, 512)], st[:]
                            )

    nc.compile()
    return nc


_NC = None


def _get_nc():
    global _NC
    if _NC is None:
        _NC = build_nc()
    return _NC


def make_in_maps(query, key, value, key_padding_mask, Wq, Wk, Wv, Wo, bo):
    query = np.asarray(query, dtype=np.float16)
    key = np.asarray(key, dtype=np.float16)
    value = np.asarray(value, dtype=np.float16)
    mask = np.asarray(key_padding_mask)
    wq_t = np.ascontiguousarray(np.asarray(Wq, dtype=np.float16).T)
    wk_t = np.ascontiguousarray(np.asarray(Wk, dtype=np.float16).T)
    wv_t = np.ascontiguousarray(np.asarray(Wv, dtype=np.float16).T)
    wo_t = np.ascontiguousarray(np.asarray(Wo, dtype=np.float16).T)
    bo_rep = np.ascontiguousarray(
        np.broadcast_to(np.asarray(bo, dtype=np.float32), (P, D))
    )
    in_maps = []
    for core in range(8):
        b, jq = core // 2, core % 2
        in_maps.append(
            {
                "qT": np.ascontiguousarray(query[b, jq * SQ : (jq + 1) * SQ, :].T),
                "kT": np.ascontiguousarray(key[b].T),
                "vT": np.ascontiguousarray(value[b].T),
                "wq": wq_t,
                "wk": wk_t,
                "wv": wv_t,
                "wo": wo_t,
                "bo": bo_rep,
                "mask": np.ascontiguousarray(
                    mask[b].astype(np.float32).reshape(NKT, P).T
                ),
            }
        )
    return in_maps


def run_sharded(inputs, trace=False, trace_cores=None):
    nc = _get_nc()
    in_maps = make_in_maps(**inputs)
    res = run_bass_kernel_spmd(
        nc,
        in_maps,
        list(range(8)),
        trace=trace,
        trace_cores=trace_cores,
    )
    full = np.empty((B, S, D), dtype=np.float32)
    for core in range(8):
        b, jq = core // 2, core % 2
        full[b, jq * SQ : (jq + 1) * SQ, :] = res.results[core]["out"]
    return full, res


def kernel(**inputs):
    full, _ = run_sharded(inputs)
    return full
